# revision 3
# baseline (speedup 1.0000x reference)
"""Multi-head self-attention (B=2, S=2048, D=1024, H=16) on 8 Trainium2 cores.

Sharding: core c handles batch c//4 and head-group c%4 (4 heads = 256 channels).
Per-core device program (SPMD, different data per core):
  phase A: Q/K projections fp32, V projection fp16, V transposed on PE
  phase B: scoresT = K_blk^T Q (row-packed fp32 pairs) -> exp (fp32, ACT)
           -> fp16 copy (GpSimd) -> Z via ones-matmul + ctxT via V-stationary
           matmul (both fp16, col-packed) -> reciprocal -> partition broadcast
           -> in-place normalize (DVE/GpSimd) -> DMA attn^T to HBM
  phase C: out partial = ctxT^T @ Wo_slice^T in fp16, interleaved per q-group
Host: shards/transposes inputs, sums the 8 output partials + bo, and
transposes attn^T -> attn.
"""
import sys
sys.path.insert(0, "/opt/trn_rl_repo")
import numpy as np

B, S, D, H = 2, 2048, 1024, 16
HD = D // H  # 64
NCORES = 8
GROUPS = 4          # head-groups (one per core within a batch)
GCH = D // GROUPS   # 256 channels per group
NCH = 2             # 128-partition channel chunks per group
NDIN = D // 128     # 8 input-dim chunks
NQG = 4             # q groups of 512
QG = S // NQG       # 512
NKC = S // 128      # 16 k chunks

ET_BUFS = 18
_CK = None


def _build_nc():
    import concourse.mybir as mybir
    import concourse.tile as tile
    from concourse import bacc
    from concourse.masks import make_identity

    f32 = mybir.dt.float32
    f16 = mybir.dt.float16
    AF = mybir.ActivationFunctionType
    AL = mybir.AluOpType

    nc = bacc.Bacc(None, target_bir_lowering=False)

    xt_h = nc.dram_tensor("xt", [D, S], f32, kind="ExternalInput")
    xth_h = nc.dram_tensor("xth", [D, S], f16, kind="ExternalInput")
    wqt_h = nc.dram_tensor("wqt", [D, GCH], f32, kind="ExternalInput")
    wkt_h = nc.dram_tensor("wkt", [D, GCH], f32, kind="ExternalInput")
    wvh_h = nc.dram_tensor("wvh", [D, GCH], f16, kind="ExternalInput")
    woh_h = nc.dram_tensor("woh", [GCH, D], f16, kind="ExternalInput")
    bq2_h = nc.dram_tensor("bq2", [128, NCH], f32, kind="ExternalInput")
    bk2_h = nc.dram_tensor("bk2", [128, NCH], f32, kind="ExternalInput")
    bv2_h = nc.dram_tensor("bv2", [128, NCH], f32, kind="ExternalInput")

    attnT_h = nc.dram_tensor("attnT", [4, S, S], f32, kind="ExternalOutput")
    outp_h = nc.dram_tensor("outp", [S, D], f32, kind="ExternalOutput")

    with tile.TileContext(nc) as tc:
        with tc.tile_pool(name="persist", bufs=1) as pp:
            # ---- persistent tiles ----
            qt = [pp.tile([128, S], f32, tag=f"qt{i}", name=f"qt{i}") for i in range(NCH)]
            kt = [pp.tile([128, S], f32, tag=f"kt{i}", name=f"kt{i}") for i in range(NCH)]
            vbf = pp.tile([128, NKC * GCH], f16, tag="vbf")  # [tok128, kc*256+ch]
            ctxh = [pp.tile([128, S], f16, tag=f"ctxh{i}", name=f"ctxh{i}") for i in range(NCH)]
            woh = [pp.tile([128, D], f16, tag=f"woh{i}", name=f"woh{i}") for i in range(NCH)]
            onesh = pp.tile([128, 1], f16, tag="onesh")
            identh = pp.tile([128, 128], f16, tag="identh")
            bq2 = pp.tile([128, NCH], f32, tag="bq2")
            bk2 = pp.tile([128, NCH], f32, tag="bk2")
            bv2 = pp.tile([128, NCH], f32, tag="bv2")

            nc.sync.dma_start(bq2[:], bq2_h[:])
            nc.sync.dma_start(bk2[:], bk2_h[:])
            nc.sync.dma_start(bv2[:], bv2_h[:])
            for i in range(NCH):
                nc.sync.dma_start(woh[i][:], woh_h[i * 128:(i + 1) * 128, :])

            onesf = pp.tile([128, 1], f32, tag="onesf")
            nc.vector.memset(onesf[:], 1.0)
            nc.vector.tensor_copy(onesh[:], onesf[:])
            make_identity(nc, identh[:])

            # ---- phase A: projections ----
            with (
                tc.tile_pool(name="pa", bufs=1) as pa,
                tc.tile_pool(name="psa", bufs=4, space="PSUM") as psa,
                tc.tile_pool(name="psat", bufs=2, space="PSUM") as psat,
            ):
                xt = [pa.tile([128, S], f32, tag=f"xt{d}", name=f"xtt{d}") for d in range(NDIN)]
                xth = [pa.tile([128, S], f16, tag=f"xth{d}", name=f"xtht{d}") for d in range(NDIN)]
                wqt = [pa.tile([128, GCH], f32, tag=f"wqt{d}", name=f"wqtt{d}") for d in range(NDIN)]
                wkt = [pa.tile([128, GCH], f32, tag=f"wkt{d}", name=f"wktt{d}") for d in range(NDIN)]
                wvh = [pa.tile([128, GCH], f16, tag=f"wvh{d}", name=f"wvht{d}") for d in range(NDIN)]
                vth = [pa.tile([128, S], f16, tag=f"vth{i}", name=f"vtht{i}") for i in range(NCH)]

                for d in range(NDIN):
                    sl = slice(d * 128, (d + 1) * 128)
                    nc.sync.dma_start(xt[d][:], xt_h[sl, :])
                    nc.sync.dma_start(xth[d][:], xth_h[sl, :])
                    nc.sync.dma_start(wqt[d][:], wqt_h[sl, :])
                    nc.sync.dma_start(wkt[d][:], wkt_h[sl, :])
                    nc.sync.dma_start(wvh[d][:], wvh_h[sl, :])

                # Q and K in fp32; V in fp16
                for wts, dst, bias, scale, xsrc in (
                    (wqt, qt, bq2, 0.125, xt),
                    (wkt, kt, bk2, 1.0, xt),
                    (wvh, vth, bv2, 1.0, xth),
                ):
                    for ch in range(NCH):
                        chsl = slice(ch * 128, (ch + 1) * 128)
                        for tg in range(NQG):
                            tsl = slice(tg * QG, (tg + 1) * QG)
                            ps = psa.tile([128, QG], f32, tag="pA")
                            for d in range(NDIN):
                                nc.tensor.matmul(
                                    ps[:], wts[d][:, chsl], xsrc[d][:, tsl],
                                    start=(d == 0), stop=(d == NDIN - 1),
                                )
                            nc.scalar.activation(
                                dst[ch][:, tsl], ps[:], AF.Identity,
                                bias=bias[:, ch:ch + 1], scale=scale,
                            )

                # transpose V: vth [128ch, tok] -> vbf [128tok, kc*256+ch]
                for ch in range(NCH):
                    for tb in range(NKC):
                        pt = psat.tile([128, 128], f16, tag="pT")
                        nc.tensor.transpose(
                            pt[:], vth[ch][:, tb * 128:(tb + 1) * 128], identh[:]
                        )
                        nc.vector.tensor_copy(
                            vbf[:, tb * GCH + ch * 128: tb * GCH + (ch + 1) * 128],
                            pt[:],
                        )

            # ---- phases B and C ----
            with (
                tc.tile_pool(name="pbe", bufs=ET_BUFS) as pbe,
                tc.tile_pool(name="pbm", bufs=2) as pbm,
                tc.tile_pool(name="pbo", bufs=3) as pbo,
                tc.tile_pool(name="pssc", bufs=2, space="PSUM") as pssc,
                tc.tile_pool(name="psc", bufs=2, space="PSUM") as psc,
                tc.tile_pool(name="psz", bufs=2, space="PSUM") as psz,
            ):
                for qg in range(NQG):
                    qsl = slice(qg * QG, (qg + 1) * QG)
                    for hp in range(NCH):  # head pair = channel chunk
                        et_tiles = []
                        cpx = psc.tile([128, QG], f32, tag="cpx")
                        zp0 = psz.tile([128, QG], f32, tag="zp")
                        zp1 = psz.tile([128, QG], f32, tag="zp")
                        for kc in range(NKC):
                            ksl = slice(kc * 128, (kc + 1) * 128)
                            sc = pssc.tile([128, 2 * QG], f32, tag="sc")
                            nc.tensor.matmul(
                                sc[:, 0:QG], kt[hp][0:64, ksl], qt[hp][0:64, qsl],
                                start=True, stop=True, tile_position=(0, 0),
                            )
                            nc.tensor.matmul(
                                sc[:, QG:], kt[hp][64:128, ksl], qt[hp][64:128, qsl],
                                start=True, stop=True, tile_position=(64, 0),
                            )
                            et = pbe.tile([128, 2 * QG], f32, tag="et")
                            nc.scalar.activation(
                                et[:], sc[:], AF.Exp, bias=0.0, scale=1.0
                            )
                            eth = pbe.tile([128, 2 * QG], f16, tag="eth")
                            nc.gpsimd.tensor_copy(eth[:], et[:])
                            et_tiles.append((et, eth))
                            # Z sums (fp16 ones-matmul), array col groups 0 / 32
                            nc.tensor.matmul(
                                zp0[0:1, :], onesh[:], eth[:, 0:QG],
                                start=(kc == 0), stop=(kc == NKC - 1),
                                tile_position=(0, 0),
                            )
                            nc.tensor.matmul(
                                zp1[32:33, :], onesh[:], eth[:, QG:],
                                start=(kc == 0), stop=(kc == NKC - 1),
                                tile_position=(0, 32),
                            )
                            # ctxT accumulation, col-packed heads
                            vco = kc * GCH + hp * 128
                            nc.tensor.matmul(
                                cpx[0:64, :], vbf[:, vco:vco + 64], eth[:, 0:QG],
                                start=(kc == 0), stop=False, tile_position=(0, 0),
                            )
                            nc.tensor.matmul(
                                cpx[64:128, :], vbf[:, vco + 64:vco + 128],
                                eth[:, QG:],
                                start=False, stop=(kc == NKC - 1),
                                tile_position=(0, 64),
                            )
                        # softmax denominators
                        zr = pbm.tile([1, 2 * QG], f32, tag="zr")
                        nc.vector.reciprocal(zr[:, 0:QG], zp0[0:1, :])
                        nc.vector.reciprocal(zr[:, QG:], zp1[32:33, :])
                        bc = pbm.tile([128, 2 * QG], f32, tag="bc")
                        nc.gpsimd.partition_broadcast(bc[:], zr[:])
                        # normalize in place + write attnT
                        for kc in range(NKC):
                            et, eth = et_tiles[kc]
                            eng = nc.vector if (kc % 2 == 0) else nc.gpsimd
                            eng.tensor_tensor(
                                out=et[:], in0=et[:], in1=bc[:], op=AL.mult
                            )
                            ksl = slice(kc * 128, (kc + 1) * 128)
                            nc.sync.dma_start(
                                attnT_h[2 * hp, ksl, qsl], et[:, 0:QG]
                            )
                            nc.sync.dma_start(
                                attnT_h[2 * hp + 1, ksl, qsl], et[:, QG:]
                            )
                        # normalized ctxT in fp16
                        nc.vector.tensor_tensor(
                            out=ctxh[hp][0:64, qsl], in0=cpx[0:64, :],
                            in1=bc[0:64, 0:QG], op=AL.mult,
                        )
                        nc.vector.tensor_tensor(
                            out=ctxh[hp][64:128, qsl], in0=cpx[64:128, :],
                            in1=bc[64:128, QG:], op=AL.mult,
                        )

                    # ---- phase C for this q-group's 4 token blocks ----
                    for tb in range(qg * 4, (qg + 1) * 4):
                        tsl = slice(tb * 128, (tb + 1) * 128)
                        osb = pbo.tile([128, D], f32, tag="osb")
                        for oh in range(2):
                            osl = slice(oh * 512, (oh + 1) * 512)
                            po = psc.tile([128, 512], f32, tag="cpx")
                            for ch in range(NCH):
                                nc.tensor.matmul(
                                    po[:], ctxh[ch][:, tsl], woh[ch][:, osl],
                                    start=(ch == 0), stop=(ch == NCH - 1),
                                )
                            nc.scalar.copy(osb[:, osl], po[:])
                        nc.sync.dma_start(outp_h[tsl, :], osb[:])
    nc.compile()
    return nc


class _Compiled:
    """Compile once; rerun cheaply. Mirrors bass2jax.run_bass_via_pjrt."""

    def __init__(self, nc, n_cores=NCORES):
        import jax
        import concourse.mybir as mybir
        from jax.sharding import Mesh, PartitionSpec
        from jax.experimental.shard_map import shard_map
        from concourse.bass2jax import (
            _bass_exec_p, install_neuronx_cc_hook, partition_id_tensor,
        )

        install_neuronx_cc_hook()
        self.jax = jax
        self.n_cores = n_cores
        partition_name = (
            nc.partition_id_tensor.name if nc.partition_id_tensor else None
        )
        in_names, out_names, out_avals, zero_outs = [], [], [], []
        for alloc in nc.m.functions[0].allocations:
            if not isinstance(alloc, mybir.MemoryLocationSet):
                continue
            name = alloc.memorylocations[0].name
            if alloc.kind == "ExternalInput":
                if name != partition_name:
                    in_names.append(name)
            elif alloc.kind == "ExternalOutput":
                out_names.append(name)
                shape = tuple(alloc.tensor_shape)
                dtype = mybir.dt.np(alloc.dtype)
                out_avals.append(jax.core.ShapedArray(shape, dtype))
                zero_outs.append(np.zeros(shape, dtype))
        self.in_names, self.out_names = in_names, out_names
        self.out_avals, self.zero_outs = out_avals, zero_outs
        n_params, n_outs = len(in_names), len(out_avals)
        all_in = list(in_names) + list(out_names)
        if partition_name is not None:
            all_in.append(partition_name)

        def _body(*args):
            operands = list(args)
            if partition_name is not None:
                operands.append(partition_id_tensor())
            return tuple(
                _bass_exec_p.bind(
                    *operands,
                    out_avals=tuple(out_avals),
                    in_names=tuple(all_in),
                    out_names=tuple(out_names),
                    lowering_input_output_aliases=(),
                    sim_require_finite=False,
                    sim_require_nnan=False,
                    nc=nc,
                )
            )

        devices = jax.devices()[:n_cores]
        self.mesh = Mesh(np.asarray(devices), ("core",))
        self.fn = jax.jit(
            shard_map(
                _body, mesh=self.mesh,
                in_specs=(PartitionSpec("core"),) * (n_params + n_outs),
                out_specs=(PartitionSpec("core"),) * n_outs,
                check_rep=False,
            ),
            keep_unused=True,
        )

    def device_args(self, in_maps):
        jax = self.jax
        from jax.sharding import NamedSharding, PartitionSpec

        n = self.n_cores
        packed = [
            np.concatenate(
                [np.asarray(in_maps[c][name]) for c in range(n)], axis=0
            )
            for name in self.in_names
        ] + [
            np.zeros((n * z.shape[0], *z.shape[1:]), z.dtype)
            for z in self.zero_outs
        ]
        sh = NamedSharding(self.mesh, PartitionSpec("core"))
        return [jax.device_put(a, sh) for a in packed]

    def run(self, in_maps):
        jax = self.jax
        args = self.device_args(in_maps)
        out = self.fn(*args)
        jax.block_until_ready(out)
        n = self.n_cores
        return [
            {
                name: np.asarray(out[i]).reshape(n, *self.out_avals[i].shape)[c]
                for i, name in enumerate(self.out_names)
            }
            for c in range(n)
        ]


def _get_compiled():
    global _CK
    if _CK is None:
        _CK = _Compiled(_build_nc())
    return _CK


def make_in_maps(x, Wq, bq, Wk, bk, Wv, bv, Wo, bo):
    x = np.asarray(x, np.float32)
    xts, xths = [], []
    for b in range(B):
        xt = np.ascontiguousarray(x[b].T)
        xts.append(xt)
        xths.append(xt.astype(np.float16))
    in_maps = []
    for c in range(NCORES):
        b, g = c // GROUPS, c % GROUPS
        chs = slice(g * GCH, (g + 1) * GCH)
        in_maps.append({
            "xt": xts[b],
            "xth": xths[b],
            "wqt": np.ascontiguousarray(np.asarray(Wq, np.float32)[chs, :].T),
            "wkt": np.ascontiguousarray(np.asarray(Wk, np.float32)[chs, :].T),
            "wvh": np.ascontiguousarray(
                np.asarray(Wv, np.float32)[chs, :].T
            ).astype(np.float16),
            "woh": np.ascontiguousarray(
                np.asarray(Wo, np.float32)[:, chs].T
            ).astype(np.float16),
            "bq2": np.asarray(bq, np.float32)[chs].reshape(NCH, 128).T
            * np.float32(0.125),
            "bk2": np.ascontiguousarray(
                np.asarray(bk, np.float32)[chs].reshape(NCH, 128).T
            ),
            "bv2": np.ascontiguousarray(
                np.asarray(bv, np.float32)[chs].reshape(NCH, 128).T
            ),
        })
    return in_maps


def kernel(x, Wq, bq, Wk, bk, Wv, bv, Wo, bo):
    ck = _get_compiled()
    results = ck.run(make_in_maps(x, Wq, bq, Wk, bk, Wv, bv, Wo, bo))

    out = np.zeros((B, S, D), np.float32)
    attn = np.empty((B, H, S, S), np.float32)
    for c in range(NCORES):
        b, g = c // GROUPS, c % GROUPS
        out[b] += results[c]["outp"]
        attn[b, 4 * g:4 * g + 4] = results[c]["attnT"].swapaxes(1, 2)
    out += np.asarray(bo, np.float32)[None, None, :]
    return out, attn
